# revision 37
# baseline (speedup 1.0000x reference)
"""LoRA Multihead Attention on 8 TRN2 NeuronCores.

Sharding: tensor-parallel over heads. Core c owns heads {2c, 2c+1}
(= channel slice [128c, 128c+128)). Per core:
  1. q,k projections (feature-major, fp16); v projection (token-major)
     with 2 extra "c-channel" outputs c_h(j) = SCALE*bq_h.(Wk_h x_j),
  2. attention S^T = k^T q (2 heads row-tiled concurrently on the PE),
     P = exp(S^T + c) with the per-key bias c applied inside the exp:
     most tiles on ACT (exact exp, per-partition bias AP), a fraction on
     DVE via the Schraudolph int16 bit-trick writing fp16 bit patterns,
  3. PV with a ones column appended to V (deferred softmax denominator);
     normalization via DVE reciprocal + a K=1 PE matmul that broadcasts
     the reciprocal across partitions (keeps the Pool engine free),
  4. partial out_proj: each core computes Wout_eff[:, own 128 ch] @ osb
     (full E rows) from local data -- no gather needed -- then a chunked
     f32 ReduceScatter sums partials and leaves each core its own
     128-row slice, written directly to the kernel output.

LoRA and the v-bias are folded into Wout_eff/bout_eff on the host
(W_eff = Wout + 2*B@A; bias_eff = b + W_eff@bv, added on host). The
k-bias drops by softmax invariance; the q-bias survives only via c.
All matmuls fp16 with fp32 PSUM accumulation; collective reduces f32.
"""

import sys

sys.path.insert(0, "/opt/trn_rl_repo")

import numpy as np

import concourse.bass as bass  # noqa: F401  (import keeps bass registered)
import concourse.tile as tile
from concourse import bacc, mybir
from concourse.bass_utils import run_bass_kernel_spmd

F16 = np.float16
f16 = mybir.dt.float16
i16 = mybir.dt.int16
f32 = mybir.dt.float32

L, N, E = 2048, 2, 1024
T = N * L            # 4096 tokens, t = n*L + l
H, D = 16, 64
NCORES = 8
HPC = H // NCORES    # heads per core = 2
CS = HPC * D         # channel slice width per core = 128
SCALE = D ** -0.5
LORA_SCALING = 32.0 / 16.0

LB = 512             # l-block (moving free dim)
NT = T // LB         # 8 t-blocks over all tokens
NLB = L // LB        # 4 l-blocks per batch
NMT = L // 128       # 16 key-tiles per batch
NE = E // 128        # 8 contraction tiles
VW = CS + 2          # v-proj output width (128 v channels + 2 c channels)
# ReduceScatter chunks as (n, lb_start, lb_end), one per lb-pair
CHUNKS = ((0, 0, 2), (0, 2, 4), (1, 0, 2), (1, 2, 4))
NCH = len(CHUNKS)

# fp16 Schraudolph bit-trick: int16 = round(s*C1 + C0) viewed as fp16
C1 = 1024.0 / np.log(2.0)
C0 = 15.0 * 1024.0 - 58.7
def exp_on_dve(mt, h):
    # 14 of 32 exps per chunk go to DVE so ACT/DVE rates match the PE rate
    return (((mt % 2 == 1) and (h == 1)) or (mt % 4 == 1 and h == 0)
            or (mt % 8 == 6 and h == 1))

_CACHE = {}


def _build_nc(reps=1):
    nc = bacc.Bacc("TRN2", target_bir_lowering=False, debug=False,
                   enable_asserts=False, num_devices=NCORES)

    qT_d = nc.dram_tensor("qT", [E, T], f16, kind="ExternalInput")
    wqkt_d = nc.dram_tensor("wqkt", [E, 2 * CS], f16, kind="ExternalInput")
    wvt_d = nc.dram_tensor("wvt", [E, VW], f16, kind="ExternalInput")
    wort_d = nc.dram_tensor("wort", [CS, E], f16, kind="ExternalInput")

    rs_in = [nc.dram_tensor(f"rsi{k}", [E, (c[2] - c[1]) * LB], f16)
             for k, c in enumerate(CHUNKS)]
    rs_mid = [nc.dram_tensor(f"rsm{k}", [CS, (c[2] - c[1]) * LB], f16)
              for k, c in enumerate(CHUNKS)]
    rs_out = [nc.dram_tensor(f"rso{k}", [CS, (c[2] - c[1]) * LB], f16,
                             kind="ExternalOutput")
              for k, c in enumerate(CHUNKS)]

    with tile.TileContext(nc) as tc:
        with (
            tc.tile_pool(name="const", bufs=1) as cp,
            tc.tile_pool(name="qt", bufs=1) as qtp,
            tc.tile_pool(name="qks", bufs=1) as qksp,
            tc.tile_pool(name="vp", bufs=1) as vp,
            tc.tile_pool(name="pp", bufs=8) as pp,
            tc.tile_pool(name="osb", bufs=1) as osbp,
            tc.tile_pool(name="small", bufs=4) as smp,
            tc.tile_pool(name="fsb", bufs=4) as fsp,
            tc.tile_pool(name="ps_sp", bufs=2, space="PSUM") as ps_sp,
            tc.tile_pool(name="ps_o", bufs=4, space="PSUM") as ps_o,
        ):
            # ---- load constants & qT (qT chunked per (e, tb) for early start)
            wqkt = [cp.tile([128, 2 * CS], f16, tag=f"wqkt{e}", name=f"wqkt{e}") for e in range(NE)]
            wvt = [cp.tile([128, VW], f16, tag=f"wvt{e}", name=f"wvt{e}") for e in range(NE)]
            wort = cp.tile([CS, E], f16, tag="wort", name="wort")
            ones1 = cp.tile([1, D], f16, tag="ones1", name="ones1")
            nc.vector.memset(ones1[:], 1.0)
            qt = [qtp.tile([128, T], f16, tag=f"qt{e}", name=f"qt{e}") for e in range(NE)]
            for e in range(NE):
                sl = slice(e * 128, (e + 1) * 128)
                eng = nc.sync if e % 2 == 0 else nc.scalar
                eng.dma_start(wqkt[e][:], wqkt_d.ap()[sl, :])
            for tb in range(NT):
                cs = slice(tb * LB, (tb + 1) * LB)
                for e in range(NE):
                    sl = slice(e * 128, (e + 1) * 128)
                    eng = nc.sync if e % 2 == 0 else nc.scalar
                    eng.dma_start(qt[e][:, cs], qT_d.ap()[sl, cs])
                if tb == 0:
                    for e in range(NE):
                        sl = slice(e * 128, (e + 1) * 128)
                        eng = nc.sync if e % 2 == 0 else nc.scalar
                        eng.dma_start(wvt[e][:], wvt_d.ap()[sl, :])
                    nc.sync.dma_start(wort[:], wort_d.ap())

            for _rep in range(reps):
              # ---- q,k projection: qks[ch] = W_{q|k,slice} @ query^T, fp16
              qks = [qksp.tile([128, T], f16, tag=f"qks{ch}", name=f"qks{ch}") for ch in range(2)]
              for ch in range(2):
                  for tb in range(NT):
                      pm = ps_o.tile([128, LB], f32, tag="acc", name="pm")
                      cs = slice(tb * LB, (tb + 1) * LB)
                      for e in range(NE):
                          nc.tensor.matmul(pm[:], wqkt[e][:, ch * CS:(ch + 1) * CS],
                                           qt[e][:, cs], start=(e == 0), stop=(e == NE - 1))
                      nc.vector.tensor_copy(qks[ch][:, cs], pm[:])

              # ---- v projection, token-major with ones column: v_all[n][h] (128, 16*65)
              # plus 2 c-channels per token-tile staged into cstage
              v_all = [[vp.tile([128, NMT * (D + 1)], f16, tag=f"v{n}{h}", name=f"v{n}{h}")
                        for h in range(2)] for n in range(N)]
              cstage = vp.tile([128, 2 * T // 128], f32, tag="cst", name="cstage")
              dstage = vp.tile([128, 2 * T // 128], f32, tag="dst", name="dstage")
              for n in range(N):
                  for h in range(2):
                      # ones columns at 64::65 via one strided memset
                      nc.vector.memset(v_all[n][h][:, D::D + 1], 1.0)
              for mt in range(T // 128):
                  pm = ps_o.tile([128, VW], f32, tag="acc", name="pmv")
                  cs = slice(mt * 128, (mt + 1) * 128)
                  for e in range(NE):
                      nc.tensor.matmul(pm[:], qt[e][:, cs], wvt[e][:],
                                       start=(e == 0), stop=(e == NE - 1))
                  n, mti = mt // NMT, mt % NMT
                  for h in range(2):
                      nc.vector.tensor_copy(
                          v_all[n][h][:, mti * (D + 1):mti * (D + 1) + D],
                          pm[:, h * D:(h + 1) * D])
                  nc.vector.tensor_copy(cstage[:, 2 * mt:2 * mt + 2], pm[:, CS:VW])
              # dstage = C1*c + C0 for the DVE bit-trick tiles
              nc.vector.tensor_scalar(dstage[:], cstage[:], float(C1), float(C0),
                                      mybir.AluOpType.mult, mybir.AluOpType.add)

              # ---- attention (heads paired for PE row-group concurrency),
              # with partial out_proj + ReduceScatter per lb-pair chunk ----
              osb = [osbp.tile([CS, L], f16, tag=f"osb{n}", name=f"osb{n}")
                     for n in range(N)]
              for k, (n, lb0, lb1) in enumerate(CHUNKS):
                  base = n * L
                  ls = slice(base + lb0 * LB, base + lb1 * LB)      # 1024 queries
                  lsl = slice(lb0 * LB, lb1 * LB)
                  # o_ps[h][half] accumulators, 1 bank each
                  o_ps = [[ps_o.tile([D + 1, LB], f32, tag="acc", name="ops")
                           for _ in range(2)] for _ in range(2)]
                  # software-pipelined: PV for tile (mt-1, h) issues after the
                  # scores of (mt, h), so the in-order PE queue never blocks
                  # at a PV whose exp is still in flight
                  pend = []   # (mt, h, pt)
                  for mt in range(NMT):
                      ms = slice(base + mt * 128, base + (mt + 1) * 128)
                      for h in range(2):
                          d0 = h * D
                          col = 2 * (n * NMT + mt) + h
                          # score pair tile: [128 keys, 1024 queries], 2 banks
                          sp = ps_sp.tile([128, 2 * LB], f32, tag="s", name="sp")
                          for half in range(2):
                              lsh = slice(base + (lb0 + half) * LB,
                                          base + (lb0 + half + 1) * LB)
                              nc.tensor.matmul(sp[:, half * LB:(half + 1) * LB],
                                               qks[1][d0:d0 + D, ms],
                                               qks[0][d0:d0 + D, lsh],
                                               start=True, stop=True)
                          pt = pp.tile([128, 2 * LB], f16, tag="p", name="pt")
                          if exp_on_dve(mt, h):
                              nc.vector.tensor_scalar(
                                  pt[:].bitcast(i16), sp[:], float(C1),
                                  dstage[:, col:col + 1],
                                  mybir.AluOpType.mult, mybir.AluOpType.add)
                          else:
                              nc.scalar.activation(
                                  pt[:], sp[:],
                                  mybir.ActivationFunctionType.Exp,
                                  bias=cstage[:, col:col + 1], scale=1.0)
                          pend.append((mt, h, pt))
                          if len(pend) > 1:
                              pmt, ph, ppt = pend.pop(0)
                              vs = slice(pmt * (D + 1), pmt * (D + 1) + D + 1)
                              for half in range(2):
                                  nc.tensor.matmul(
                                      o_ps[ph][half][:], v_all[n][ph][:, vs],
                                      ppt[:, half * LB:(half + 1) * LB],
                                      start=(pmt == 0), stop=(pmt == NMT - 1))
                  for pmt, ph, ppt in pend:
                      vs = slice(pmt * (D + 1), pmt * (D + 1) + D + 1)
                      for half in range(2):
                          nc.tensor.matmul(
                              o_ps[ph][half][:], v_all[n][ph][:, vs],
                              ppt[:, half * LB:(half + 1) * LB],
                              start=(pmt == 0), stop=(pmt == NMT - 1))
                  rr_ps = [None, None]
                  for half in range(2):
                      for h in range(2):
                          if half == 0:
                              rr_ps[h] = ps_sp.tile([D, 2 * LB], f32, tag="s",
                                                    name="rrps")
                          rs = smp.tile([1, LB], f16, tag="rs", name="rs")
                          with nc.allow_low_precision(reason="1/denom in fp16 is plenty"):
                              nc.vector.reciprocal(rs[:], o_ps[h][half][D:D + 1, :])
                          nc.tensor.matmul(rr_ps[h][:, half * LB:(half + 1) * LB],
                                           ones1[:], rs[:], start=True, stop=True)
                          rr = smp.tile([D, LB], f16, tag="rr", name="rr")
                          nc.vector.tensor_copy(rr[:], rr_ps[h][:, half * LB:(half + 1) * LB])
                          nc.vector.tensor_mul(
                              osb[n][h * D:(h + 1) * D,
                                     (lb0 + half) * LB:(lb0 + half + 1) * LB],
                              o_ps[h][half][0:D, :], rr[:])
                      # partial out_proj for this half right away
                      osl = slice((lb0 + half) * LB, (lb0 + half + 1) * LB)
                      for m in range(NE):
                          msl = slice(m * 128, (m + 1) * 128)
                          f_ps = ps_o.tile([128, LB], f32, tag="acc", name="fps")
                          nc.tensor.matmul(f_ps[:], wort[:, msl],
                                           osb[n][:, osl],
                                           start=True, stop=True)
                          f_sb = fsp.tile([128, LB], f16, tag="fsb", name="fsb")
                          if m % 2 == 0:
                              nc.vector.tensor_copy(f_sb[:], f_ps[:])
                          else:
                              nc.scalar.copy(f_sb[:], f_ps[:])
                          eng = nc.sync if m % 2 == 0 else nc.scalar
                          eng.dma_start(
                              rs_in[k].ap()[msl, half * LB:(half + 1) * LB],
                              f_sb[:])
                  nc.gpsimd.collective_compute(
                      "ReduceScatter", mybir.AluOpType.add,
                      ins=[rs_in[k].ap()], outs=[rs_mid[k].ap()],
                      replica_groups=[list(range(NCORES))],
                  )
                  nc.sync.dma_start(rs_out[k].ap(), rs_mid[k].ap())

    nc.compile()
    return nc


def _host_prep(inputs):
    q = np.asarray(inputs["query"], np.float32)
    W = np.asarray(inputs["in_proj_weight"], np.float32)
    b = np.asarray(inputs["in_proj_bias"], np.float32)
    Wout = np.asarray(inputs["out_proj_weight"], np.float32)
    bout = np.asarray(inputs["out_proj_bias"], np.float32)
    A = np.asarray(inputs["lora_A"], np.float32)
    B = np.asarray(inputs["lora_B"], np.float32)

    qT = np.ascontiguousarray(q.transpose(2, 1, 0).reshape(E, T)).astype(F16)
    bv = b[2 * E:3 * E]
    Wout_eff = Wout + LORA_SCALING * (B @ A)
    bout_eff = bout + Wout_eff @ bv

    in_maps = []
    for c in range(NCORES):
        hs = slice(CS * c, CS * (c + 1))
        wq = W[hs, :] * SCALE
        wk = W[E + CS * c:E + CS * (c + 1), :]
        wv = W[2 * E + CS * c:2 * E + CS * (c + 1), :]
        bq = b[hs]
        # c-channel weights: SCALE * Wk_h^T @ bq_h per head
        wc = np.stack([
            SCALE * (wk[h * D:(h + 1) * D, :].T @ bq[h * D:(h + 1) * D])
            for h in range(2)], axis=1)                       # (E, 2)
        wqkt = np.ascontiguousarray(np.concatenate([wq.T, wk.T], axis=1)).astype(F16)
        wvt = np.ascontiguousarray(np.concatenate([wv.T, wc], axis=1)).astype(F16)
        in_maps.append({
            "qT": qT,
            "wqkt": wqkt,
            "wvt": wvt,
            "wort": np.ascontiguousarray(Wout_eff[:, hs].T).astype(F16),
        })
    return in_maps


def _host_post(inputs, res):
    b = np.asarray(inputs["in_proj_bias"], np.float32)
    Wout = np.asarray(inputs["out_proj_weight"], np.float32)
    bout = np.asarray(inputs["out_proj_bias"], np.float32)
    A = np.asarray(inputs["lora_A"], np.float32)
    B = np.asarray(inputs["lora_B"], np.float32)
    bv = b[2 * E:3 * E]
    Wout_eff = Wout + LORA_SCALING * (B @ A)
    bout_eff = bout + Wout_eff @ bv

    full = np.empty((E, T), np.float32)
    for c in range(NCORES):
        rows = slice(CS * c, CS * (c + 1))
        for k, (cn, lb0, lb1) in enumerate(CHUNKS):
            c0 = cn * L + lb0 * LB
            full[rows, c0:c0 + (lb1 - lb0) * LB] = np.asarray(res.results[c][f"rso{k}"], np.float32)
    full += bout_eff[:, None]
    return np.ascontiguousarray(full.reshape(E, N, L).transpose(2, 1, 0))


def _run(inputs, trace=False):
    if "nc" not in _CACHE:
        _CACHE["nc"] = _build_nc()
    nc = _CACHE["nc"]
    in_maps = _host_prep(inputs)
    res = run_bass_kernel_spmd(nc, in_maps, core_ids=list(range(NCORES)),
                               trace=trace)
    return _host_post(inputs, res), res


def kernel(**inputs):
    out, _ = _run(inputs, trace=False)
    return out
